# revision 1
# baseline (speedup 1.0000x reference)
"""Conv2D-KAN Trainium2 kernel (8-core data-parallel SPMD).

Formulation
-----------
The reference computes, per 3x3 patch (N = B*30*30 patches, in_size = 288):
    out[n,o] = sum_{i,k} sb[n,i,k] * (spline_kernel*scale)[i,k,o]
             + silu(xf) @ scale_factor + biases
where sb is a cubic B-spline basis (8 funcs) over a uniform grid
(knots t_r = -2.2 + 0.4 r, r = 0..11, h = 0.4).

Key identities:
 1. Basis values depend only on the underlying *pixel*, not the patch
    (patch extraction is a gather), so features are computed per pixel
    (8x less elementwise work than per-patch).
 2. Uniform cubic B-splines decompose over truncated powers:
        B_k(x) = (1/6) sum_{m=0..4} cm_m T_{k+m}(x), cm = [1,-4,6,-4,1]
        T_r(x) = min(relu((x - t_r)/h), 11-r)^3
    The clamp at 11-r makes every B_k *exactly* zero outside the grid
    (integer cancellation), matching the reference's out-of-range
    behaviour without masks, and T_11 == 0 so only r = 0..10 exist.
 3. The whole op is then a 3x3 convolution with 128 filters over
    pixel-feature channels, done as accumulating 128-K matmuls into
    PSUM banks of [128 filters, 450 patches].

Two modes:
 * "fp32"  — features are the 11 truncated cubes + silu per channel
             (384 = 3x128 K-chunks per offset, 27 matmuls per bank),
             blending folded into the weights. Full fp32 matmuls
             (4 cyc/row). Max rel err ~1e-5.
 * "basis" — the blending T -> B_k happens on DVE in fp32 (exact), so
             the matmul operands are the well-conditioned basis values
             (<= 4) and the matmuls run in float32r (TF32-like, 1-pass,
             ~1.4 cyc/row). 8 basis + silu -> 2x128 + 32 K-chunks per
             offset, 27 matmuls per bank. Rel err ~ a few 1e-5.

Each core processes 4 images; output [128, 3600] per core is
transposed on host.
"""

import sys

sys.path.insert(0, "/opt/trn_rl_repo")

import numpy as np

N_CORES = 8
B, HH, WW, C = 32, 32, 32, 32
F = 128
KH = KW = 3
HO, WO = HH - KH + 1, WW - KW + 1          # 30, 30
BPC = B // N_CORES                          # images per core = 4
PIX = HH * WW                               # 1024 pixels per image
NPC = BPC * HO * WO                         # 3600 patches per core
NBANK = 2 * BPC                             # 8 psum banks
BANKN = NPC // NBANK                        # 450
HGRID = 0.4
T0 = -2.2                                   # first knot
NR = 11                                     # truncated-cube features
NFEAT = 12                                  # + silu
NMM = 27                                    # matmuls per bank (both modes)

MODE = "fp32"  # "fp32" | "basis"

_cache = {}


def _build_program(mode):
    import concourse.bacc as bacc
    import concourse.mybir as mybir
    import concourse.tile as tile

    f32 = mybir.dt.float32
    f32r = mybir.dt.float32r
    AF = mybir.ActivationFunctionType
    basis = mode == "basis"

    nch = NMM + 2 if basis else NMM
    nc = bacc.Bacc("TRN2", target_bir_lowering=False, debug=False)
    xt = nc.dram_tensor("xt", [C, BPC * PIX], f32, kind="ExternalInput").ap()
    # weights: [128 partitions, nch * F] -> one contiguous DMA
    wt = nc.dram_tensor("wt", [128, nch * F], f32, kind="ExternalInput").ap()
    consts = nc.dram_tensor("consts", [128, 8], f32, kind="ExternalInput").ap()
    y = nc.dram_tensor("y", [F, NPC], f32, kind="ExternalOutput").ap()

    with tile.TileContext(nc) as tc:
        with (
            tc.tile_pool(name="wp", bufs=1) as wp,
            tc.tile_pool(name="cp", bufs=1) as cp,
            tc.tile_pool(name="fp", bufs=3) as fp,
            tc.tile_pool(name="sp", bufs=3) as sp,
            tc.tile_pool(name="op", bufs=1) as op_,
            tc.tile_pool(name="pp", bufs=4, space="PSUM") as pp,
        ):
            ct = cp.tile([128, 8], f32)
            nc.scalar.dma_start(ct[:], consts[:])

            # warm up the ACT table set (silu's set also carries relu /
            # copy / identity / square fillers) so the ~1.3us table load
            # happens before the first feature tile is ready.
            warm = cp.tile([1, 1], f32, tag="warm")
            nc.scalar.activation(warm[:], ct[:1, :1], AF.Silu)

            # image 0's first feature tile: its four replica DMAs split
            # across BOTH HWDGE queues ahead of all other traffic, so
            # the last completion semaphore (which lags ~2.5us behind
            # the data) lands as early as possible.
            ft00 = None
            if not basis:
                ft00 = fp.tile([128, PIX], f32, tag="f0")
                eng = [nc.sync, nc.scalar, nc.sync, nc.scalar]
                for rep in range(4):
                    eng[rep].dma_start(
                        ft00[32 * rep:32 * rep + 32], xt[:, 0:PIX])

            if basis:
                wbig = wp.tile([128, nch * F], f32, tag="wbig")
                nc.gpsimd.dma_start(wbig[:], wt[:])
                wrbig = wp.tile([128, NMM * F], f32r, tag="wrbig")
                nc.vector.tensor_copy(wrbig[:], wbig[:, :NMM * F])
                wtiles = [wrbig[:, i * F:(i + 1) * F] for i in range(NMM)]
                Ma = wbig[:, NMM * F:(NMM + 1) * F]
                Mb = wbig[:, (NMM + 1) * F:(NMM + 2) * F]
            else:
                # weights split into two tiles so the first 9 matmuls
                # (t-major order: all offsets of feature tile 0) only
                # depend on a small fast transfer; the big remainder
                # loads on the gpsimd queue in parallel.
                wA = wp.tile([128, 9 * F], f32, tag="wA")
                nc.scalar.dma_start(wA[:], wt[:, :9 * F])
                wB = wp.tile([128, 18 * F], f32, tag="wB")
                nc.gpsimd.dma_start(wB[:], wt[:, 9 * F:])
                wtiles = [wA[:, i * F:(i + 1) * F] for i in range(9)] + \
                         [wB[:, i * F:(i + 1) * F] for i in range(18)]

            out_t = op_.tile([F, NPC], f32)

            def banks(im, mk_rhs):
                for half in range(2):
                    ps = pp.tile([F, BANKN], f32, tag="ps")
                    k = 0
                    # t-major: the first 9 matmuls only need feature
                    # tile 0, so PE starts before tiles 1/2 are built
                    for t in range(3):
                        for off in range(KH * KW):
                            di, dj = divmod(off, KW)
                            h0 = half * 15 + di
                            lhsT, rhs = mk_rhs(off, t, h0, dj)
                            nc.tensor.matmul(
                                ps[:], lhsT, rhs,
                                start=(k == 0), stop=(k == NMM - 1),
                            )
                            k += 1
                    s = (im * 2 + half) * BANKN
                    nc.scalar.activation(
                        out_t[:, s:s + BANKN], ps[:], AF.Identity,
                        bias=ct[:, 6:7], scale=1.0,
                    )
                    nc.sync.dma_start(y[:, s:s + BANKN], out_t[:, s:s + BANKN])

            for im in range(BPC):
                sl = slice(im * PIX, (im + 1) * PIX)
                if basis:
                    # --- T tiles (same r-major 4r x 32c layout as fp32 mode)
                    Ts = []
                    for t in range(3):
                        T = fp.tile([128, PIX], f32, tag=f"T{t}")
                        for rep in range(4):
                            nc.sync.dma_start(
                                T[32 * rep:32 * rep + 32], xt[:, sl])
                        nc.scalar.activation(
                            T[:], T[:], AF.Relu,
                            bias=ct[:, t:t + 1], scale=1.0 / HGRID)
                        nc.vector.tensor_scalar_min(
                            T[:], T[:], ct[:, 3 + t:4 + t])
                        sq = sp.tile([128, PIX], f32, tag="sq")
                        nc.scalar.activation(sq[:], T[:], AF.Square)
                        nc.vector.tensor_mul(T[:], sq[:], T[:])
                        Ts.append(T)
                    # --- combine B_k = sum_m cm_m T_{k+m} on PE:
                    # two banded constant matrices contract the r dim
                    # (engines cannot read shifted partition windows).
                    Bviews = []
                    for g in range(2):
                        Bt = fp.tile([128, PIX], f32r, tag=f"B{g}")
                        for hf in range(2):
                            hs = slice(hf * 512, (hf + 1) * 512)
                            bp = pp.tile([128, 512], f32, tag="psB")
                            nc.tensor.matmul(bp[:], Ma, Ts[g][:, hs],
                                             start=True, stop=False)
                            nc.tensor.matmul(bp[:], Mb, Ts[g + 1][:, hs],
                                             start=False, stop=True)
                            nc.scalar.activation(Bt[:, hs], bp[:], AF.Copy)
                        Bviews.append(
                            Bt[:].rearrange("p (h w) -> p h w", w=WW))
                    # --- silu ---
                    xs = sp.tile([32, PIX], f32, tag="xs")
                    nc.sync.dma_start(xs[:], xt[:, sl])
                    SL = fp.tile([32, PIX], f32r, tag="SL")
                    nc.scalar.activation(SL[:], xs[:], AF.Silu)
                    slv = SL[:].rearrange("p (h w) -> p h w", w=WW)

                    def mk_rhs(off, t, h0, dj, _B=Bviews, _s=slv, _w=wtiles):
                        if t < 2:
                            return (_w[off * 3 + t],
                                    _B[t][:, h0:h0 + 15, dj:dj + WO])
                        return (_w[off * 3 + 2][0:32],
                                _s[:, h0:h0 + 15, dj:dj + WO])

                    banks(im, mk_rhs)
                else:
                    views = []
                    dma_eng = [nc.sync, nc.scalar, nc.sync]
                    for t in range(3):
                        if im == 0 and t == 0:
                            ft = ft00
                        else:
                            ft = fp.tile([128, PIX], f32, tag=f"f{t}")
                            for rep in range(4):
                                dma_eng[t].dma_start(
                                    ft[32 * rep:32 * rep + 32], xt[:, sl])
                        nsp = 128 if t < 2 else 96
                        nc.scalar.activation(
                            ft[:nsp], ft[:nsp], AF.Relu,
                            bias=ct[:nsp, t:t + 1], scale=1.0 / HGRID)
                        if t == 2:
                            nc.scalar.activation(
                                ft[96:128], ft[96:128], AF.Silu)
                        nc.vector.tensor_scalar_min(
                            ft[:nsp], ft[:nsp], ct[:nsp, 3 + t:4 + t])
                        sq = sp.tile([128, PIX], f32, tag="sq")
                        nc.vector.tensor_mul(sq[:nsp], ft[:nsp], ft[:nsp])
                        nc.vector.tensor_mul(ft[:nsp], sq[:nsp], ft[:nsp])
                        views.append(
                            ft[:].rearrange("p (h w) -> p h w", w=WW))

                    def mk_rhs(off, t, h0, dj, _v=views, _w=wtiles):
                        return (_w[t * 9 + off],
                                _v[t][:, h0:h0 + 15, dj:dj + WO])

                    banks(im, mk_rhs)

    nc.compile()
    return nc


def _prep_fp32(spline_kernel, scale_factor):
    """Truncated-power-folded weights, r-major (r, c) K layout."""
    w = spline_kernel.astype(np.float64) * scale_factor.astype(np.float64)[:, None, :]
    cm = np.array([1.0, -4.0, 6.0, -4.0, 1.0], np.float64) / 6.0
    Wp = np.zeros((KH * KW, NFEAT, C, F), np.float64)
    wr = w.reshape(KH * KW, C, 8, F)
    for r in range(NR):
        for m in range(5):
            k = r - m
            if 0 <= k < 8:
                Wp[:, r] += wr[:, :, k] * cm[m]
    Wp[:, NR] = scale_factor.astype(np.float64).reshape(KH * KW, C, F)
    Wt = Wp.reshape(KH * KW, 3, 128, F)
    # device chunk order is t-major: chunk index = t*9 + off
    return np.ascontiguousarray(Wt.transpose(1, 0, 2, 3)).reshape(NMM, 128, F)


def _prep_basis(spline_kernel, scale_factor):
    """Raw spline weights /6, (4k x 32c) K layout + silu chunks."""
    w6 = (spline_kernel.astype(np.float64)
          * scale_factor.astype(np.float64)[:, None, :]) / 6.0
    w6 = w6.reshape(KH * KW, C, 8, F)
    sf = scale_factor.astype(np.float64).reshape(KH * KW, C, F)
    Wt = np.zeros((NMM + 2, 128, F), np.float64)
    for off in range(KH * KW):
        for g in range(2):
            blk = w6[off, :, 4 * g:4 * g + 4]            # (32c, 4k, F)
            Wt[off * 3 + g] = blk.transpose(1, 0, 2).reshape(128, F)
        Wt[off * 3 + 2, 0:32] = sf[off]
    # banded combine matrices: B[p_out] = sum_in M[p_in, p_out] T[p_in]
    cm = np.array([1.0, -4.0, 6.0, -4.0, 1.0])
    pin = np.arange(128)[:, None]
    pout = np.arange(128)[None, :]
    same_c = (pin % 32) == (pout % 32)
    for j, base in ((NMM, 0), (NMM + 1, 4)):
        m = base + pin // 32 - pout // 32
        val = np.where((m >= 0) & (m <= 4) & same_c, cm[np.clip(m, 0, 4)], 0.0)
        Wt[j] = val
    return Wt


def _prep_static(mode, spline_kernel, scale_factor, kan_bias, conv_bias):
    if mode == "basis":
        Wt = _prep_basis(spline_kernel, scale_factor)
    else:
        Wt = _prep_fp32(spline_kernel, scale_factor)
    nch = Wt.shape[0]
    wt = np.ascontiguousarray(
        Wt.transpose(1, 0, 2).reshape(128, nch * F), np.float32)

    consts = np.zeros((128, 8), np.float32)
    p = np.arange(128)
    for t in range(3):
        r = 4 * t + p // 32
        consts[:, t] = -(T0 + HGRID * r) / HGRID           # 5.5 - r
        consts[:, 3 + t] = NR - r                           # 11 - r
    consts[:, 6] = (kan_bias.astype(np.float64)
                    + conv_bias.astype(np.float64)).astype(np.float32)
    return wt, consts


def kernel(x, spline_kernel, scale_factor, kan_bias, conv_bias):
    from concourse import bass_utils

    x = np.asarray(x, np.float32)
    spline_kernel = np.asarray(spline_kernel, np.float32)
    scale_factor = np.asarray(scale_factor, np.float32)
    kan_bias = np.asarray(kan_bias, np.float32)
    conv_bias = np.asarray(conv_bias, np.float32)

    key = f"nc_{MODE}"
    if key not in _cache:
        _cache[key] = _build_program(MODE)
    nc = _cache[key]

    wt, consts = _prep_static(MODE, spline_kernel, scale_factor,
                              kan_bias, conv_bias)

    in_maps = []
    for c in range(N_CORES):
        xc = x[c * BPC:(c + 1) * BPC]                      # (4,32,32,32)
        xtc = np.ascontiguousarray(
            xc.transpose(3, 0, 1, 2).reshape(C, BPC * PIX), np.float32
        )
        in_maps.append({"xt": xtc, "wt": wt, "consts": consts})

    res = bass_utils.run_bass_kernel_spmd(
        nc, in_maps, core_ids=list(range(N_CORES)),
        **_cache.get("run_kwargs", {})
    )
    _cache["last_result"] = res

    out = np.empty((B, HO, WO, F), np.float32)
    for c in range(N_CORES):
        yc = res.results[c]["y"]                           # (128, 3600)
        out[c * BPC:(c + 1) * BPC] = (
            yc.reshape(F, BPC, HO, WO).transpose(1, 2, 3, 0)
        )
    return out



# revision 2
# speedup vs baseline: 1.5751x; 1.5751x over previous
"""Conv2D-KAN Trainium2 kernel (8-core data-parallel SPMD).

Formulation
-----------
The reference computes, per 3x3 patch (N = B*30*30 patches, in_size = 288):
    out[n,o] = sum_{i,k} sb[n,i,k] * (spline_kernel*scale)[i,k,o]
             + silu(xf) @ scale_factor + biases
where sb is a cubic B-spline basis (8 funcs) over a uniform grid
(knots t_r = -2.2 + 0.4 r, r = 0..11, h = 0.4).

Key identities:
 1. Basis values depend only on the underlying *pixel*, not the patch
    (patch extraction is a gather), so features are computed per pixel
    (8x less elementwise work than per-patch).
 2. Uniform cubic B-splines decompose over truncated powers:
        B_k(x) = (1/6) sum_{m=0..4} cm_m T_{k+m}(x), cm = [1,-4,6,-4,1]
        T_r(x) = min(relu((x - t_r)/h), 11-r)^3
    The clamp at 11-r makes every B_k *exactly* zero outside the grid
    (integer cancellation), matching the reference's out-of-range
    behaviour without masks, and T_11 == 0 so only r = 0..10 exist.
 3. The whole op is then a 3x3 convolution with 128 filters over
    pixel-feature channels, done as accumulating 128-K matmuls into
    PSUM banks of [128 filters, 450 patches].

Two modes:
 * "fp32"  — features are the 11 truncated cubes + silu per channel
             (384 = 3x128 K-chunks per offset, 27 matmuls per bank),
             blending folded into the weights. Full fp32 matmuls
             (4 cyc/row). Max rel err ~1e-5.
 * "basis" — the blending T -> B_k happens on DVE in fp32 (exact), so
             the matmul operands are the well-conditioned basis values
             (<= 4) and the matmuls run in float32r (TF32-like, 1-pass,
             ~1.4 cyc/row). 8 basis + silu -> 2x128 + 32 K-chunks per
             offset, 27 matmuls per bank. Rel err ~ a few 1e-5.

Each core processes 4 images; output [128, 3600] per core is
transposed on host.
"""

import sys

sys.path.insert(0, "/opt/trn_rl_repo")

import numpy as np

N_CORES = 8
B, HH, WW, C = 32, 32, 32, 32
F = 128
KH = KW = 3
HO, WO = HH - KH + 1, WW - KW + 1          # 30, 30
BPC = B // N_CORES                          # images per core = 4
PIX = HH * WW                               # 1024 pixels per image
NPC = BPC * HO * WO                         # 3600 patches per core
NBANK = 2 * BPC                             # 8 psum banks
BANKN = NPC // NBANK                        # 450
HGRID = 0.4
T0 = -2.2                                   # first knot
NR = 11                                     # truncated-cube features
NFEAT = 12                                  # + silu
NMM = 27                                    # matmuls per bank (both modes)

MODE = "basis"  # "fp32" | "basis"

_cache = {}


def _build_program(mode):
    import concourse.bacc as bacc
    import concourse.mybir as mybir
    import concourse.tile as tile

    f32 = mybir.dt.float32
    f32r = mybir.dt.float32r
    AF = mybir.ActivationFunctionType
    basis = mode == "basis"

    nch = NMM + 2 if basis else NMM
    nc = bacc.Bacc("TRN2", target_bir_lowering=False, debug=False)
    xt = nc.dram_tensor("xt", [C, BPC * PIX], f32, kind="ExternalInput").ap()
    # weights: [128 partitions, nch * F] -> one contiguous DMA
    wt = nc.dram_tensor("wt", [128, nch * F], f32, kind="ExternalInput").ap()
    consts = nc.dram_tensor("consts", [128, 8], f32, kind="ExternalInput").ap()
    y = nc.dram_tensor("y", [F, NPC], f32, kind="ExternalOutput").ap()

    with tile.TileContext(nc) as tc:
        with (
            tc.tile_pool(name="wp", bufs=1) as wp,
            tc.tile_pool(name="cp", bufs=1) as cp,
            tc.tile_pool(name="fp", bufs=3) as fp,
            tc.tile_pool(name="sp", bufs=3) as sp,
            tc.tile_pool(name="op", bufs=1) as op_,
            tc.tile_pool(name="pp", bufs=4, space="PSUM") as pp,
        ):
            ct = cp.tile([128, 8], f32)
            nc.scalar.dma_start(ct[:], consts[:])

            # warm up the ACT table set (silu's set also carries relu /
            # copy / identity / square fillers) so the ~1.3us table load
            # happens before the first feature tile is ready.
            warm = cp.tile([1, 1], f32, tag="warm")
            nc.scalar.activation(warm[:], ct[:1, :1], AF.Silu)

            # image 0's first feature tile: its four replica DMAs split
            # across BOTH HWDGE queues ahead of all other traffic, so
            # the last completion semaphore (which lags ~2.5us behind
            # the data) lands as early as possible.
            ft00 = None
            if not basis:
                ft00 = fp.tile([128, PIX], f32, tag="f0")
                eng = [nc.sync, nc.scalar, nc.sync, nc.scalar]
                for rep in range(4):
                    eng[rep].dma_start(
                        ft00[32 * rep:32 * rep + 32], xt[:, 0:PIX])

            if basis:
                wbig = wp.tile([128, nch * F], f32, tag="wbig")
                nc.gpsimd.dma_start(wbig[:], wt[:])
                wrbig = wp.tile([128, NMM * F], f32r, tag="wrbig")
                nc.vector.tensor_copy(wrbig[:], wbig[:, :NMM * F])
                wtiles = [wrbig[:, i * F:(i + 1) * F] for i in range(NMM)]
                Ma = wbig[:, NMM * F:(NMM + 1) * F]
                Mb = wbig[:, (NMM + 1) * F:(NMM + 2) * F]
            else:
                # weights split into two tiles so the first 9 matmuls
                # (t-major order: all offsets of feature tile 0) only
                # depend on a small fast transfer; the big remainder
                # loads on the gpsimd queue in parallel.
                wA = wp.tile([128, 9 * F], f32, tag="wA")
                nc.scalar.dma_start(wA[:], wt[:, :9 * F])
                wB = wp.tile([128, 18 * F], f32, tag="wB")
                nc.gpsimd.dma_start(wB[:], wt[:, 9 * F:])
                wtiles = [wA[:, i * F:(i + 1) * F] for i in range(9)] + \
                         [wB[:, i * F:(i + 1) * F] for i in range(18)]

            out_t = op_.tile([F, NPC], f32)

            def banks(im, mk_rhs):
                for half in range(2):
                    ps = pp.tile([F, BANKN], f32, tag="ps")
                    k = 0
                    # t-major: the first 9 matmuls only need feature
                    # tile 0, so PE starts before tiles 1/2 are built
                    for t in range(3):
                        for off in range(KH * KW):
                            di, dj = divmod(off, KW)
                            h0 = half * 15 + di
                            lhsT, rhs = mk_rhs(off, t, h0, dj)
                            nc.tensor.matmul(
                                ps[:], lhsT, rhs,
                                start=(k == 0), stop=(k == NMM - 1),
                            )
                            k += 1
                    s = (im * 2 + half) * BANKN
                    nc.scalar.activation(
                        out_t[:, s:s + BANKN], ps[:], AF.Identity,
                        bias=ct[:, 6:7], scale=1.0,
                    )
                    nc.sync.dma_start(y[:, s:s + BANKN], out_t[:, s:s + BANKN])

            for im in range(BPC):
                sl = slice(im * PIX, (im + 1) * PIX)
                if basis:
                    # --- T tiles (same r-major 4r x 32c layout as fp32 mode)
                    Ts = []
                    for t in range(3):
                        T = fp.tile([128, PIX], f32, tag=f"T{t}")
                        for rep in range(4):
                            nc.sync.dma_start(
                                T[32 * rep:32 * rep + 32], xt[:, sl])
                        nc.scalar.activation(
                            T[:], T[:], AF.Relu,
                            bias=ct[:, t:t + 1], scale=1.0 / HGRID)
                        nc.vector.tensor_scalar_min(
                            T[:], T[:], ct[:, 3 + t:4 + t])
                        sq = sp.tile([128, PIX], f32, tag="sq")
                        nc.scalar.activation(sq[:], T[:], AF.Square)
                        nc.vector.tensor_mul(T[:], sq[:], T[:])
                        Ts.append(T)
                    # --- combine B_k = sum_m cm_m T_{k+m} on PE:
                    # two banded constant matrices contract the r dim
                    # (engines cannot read shifted partition windows).
                    Bviews = []
                    for g in range(2):
                        Bt = fp.tile([128, PIX], f32r, tag=f"B{g}")
                        for hf in range(2):
                            hs = slice(hf * 512, (hf + 1) * 512)
                            bp = pp.tile([128, 512], f32, tag="psB")
                            nc.tensor.matmul(bp[:], Ma, Ts[g][:, hs],
                                             start=True, stop=False)
                            nc.tensor.matmul(bp[:], Mb, Ts[g + 1][:, hs],
                                             start=False, stop=True)
                            nc.scalar.activation(Bt[:, hs], bp[:], AF.Copy)
                        Bviews.append(
                            Bt[:].rearrange("p (h w) -> p h w", w=WW))
                    # --- silu ---
                    xs = sp.tile([32, PIX], f32, tag="xs")
                    nc.sync.dma_start(xs[:], xt[:, sl])
                    SL = fp.tile([32, PIX], f32r, tag="SL")
                    nc.scalar.activation(SL[:], xs[:], AF.Silu)
                    slv = SL[:].rearrange("p (h w) -> p h w", w=WW)

                    def mk_rhs(off, t, h0, dj, _B=Bviews, _s=slv, _w=wtiles):
                        if t < 2:
                            return (_w[off * 3 + t],
                                    _B[t][:, h0:h0 + 15, dj:dj + WO])
                        return (_w[off * 3 + 2][0:32],
                                _s[:, h0:h0 + 15, dj:dj + WO])

                    banks(im, mk_rhs)
                else:
                    views = []
                    dma_eng = [nc.sync, nc.scalar, nc.sync]
                    for t in range(3):
                        if im == 0 and t == 0:
                            ft = ft00
                        else:
                            ft = fp.tile([128, PIX], f32, tag=f"f{t}")
                            for rep in range(4):
                                dma_eng[t].dma_start(
                                    ft[32 * rep:32 * rep + 32], xt[:, sl])
                        nsp = 128 if t < 2 else 96
                        nc.scalar.activation(
                            ft[:nsp], ft[:nsp], AF.Relu,
                            bias=ct[:nsp, t:t + 1], scale=1.0 / HGRID)
                        if t == 2:
                            nc.scalar.activation(
                                ft[96:128], ft[96:128], AF.Silu)
                        nc.vector.tensor_scalar_min(
                            ft[:nsp], ft[:nsp], ct[:nsp, 3 + t:4 + t])
                        sq = sp.tile([128, PIX], f32, tag="sq")
                        nc.vector.tensor_mul(sq[:nsp], ft[:nsp], ft[:nsp])
                        nc.vector.tensor_mul(ft[:nsp], sq[:nsp], ft[:nsp])
                        views.append(
                            ft[:].rearrange("p (h w) -> p h w", w=WW))

                    def mk_rhs(off, t, h0, dj, _v=views, _w=wtiles):
                        return (_w[t * 9 + off],
                                _v[t][:, h0:h0 + 15, dj:dj + WO])

                    banks(im, mk_rhs)

    nc.compile()
    return nc


def _prep_fp32(spline_kernel, scale_factor):
    """Truncated-power-folded weights, r-major (r, c) K layout."""
    w = spline_kernel.astype(np.float64) * scale_factor.astype(np.float64)[:, None, :]
    cm = np.array([1.0, -4.0, 6.0, -4.0, 1.0], np.float64) / 6.0
    Wp = np.zeros((KH * KW, NFEAT, C, F), np.float64)
    wr = w.reshape(KH * KW, C, 8, F)
    for r in range(NR):
        for m in range(5):
            k = r - m
            if 0 <= k < 8:
                Wp[:, r] += wr[:, :, k] * cm[m]
    Wp[:, NR] = scale_factor.astype(np.float64).reshape(KH * KW, C, F)
    Wt = Wp.reshape(KH * KW, 3, 128, F)
    # device chunk order is t-major: chunk index = t*9 + off
    return np.ascontiguousarray(Wt.transpose(1, 0, 2, 3)).reshape(NMM, 128, F)


def _prep_basis(spline_kernel, scale_factor):
    """Raw spline weights /6, (4k x 32c) K layout + silu chunks."""
    w6 = (spline_kernel.astype(np.float64)
          * scale_factor.astype(np.float64)[:, None, :]) / 6.0
    w6 = w6.reshape(KH * KW, C, 8, F)
    sf = scale_factor.astype(np.float64).reshape(KH * KW, C, F)
    Wt = np.zeros((NMM + 2, 128, F), np.float64)
    for off in range(KH * KW):
        for g in range(2):
            blk = w6[off, :, 4 * g:4 * g + 4]            # (32c, 4k, F)
            Wt[off * 3 + g] = blk.transpose(1, 0, 2).reshape(128, F)
        Wt[off * 3 + 2, 0:32] = sf[off]
    # banded combine matrices: B[p_out] = sum_in M[p_in, p_out] T[p_in]
    cm = np.array([1.0, -4.0, 6.0, -4.0, 1.0])
    pin = np.arange(128)[:, None]
    pout = np.arange(128)[None, :]
    same_c = (pin % 32) == (pout % 32)
    for j, base in ((NMM, 0), (NMM + 1, 4)):
        m = base + pin // 32 - pout // 32
        val = np.where((m >= 0) & (m <= 4) & same_c, cm[np.clip(m, 0, 4)], 0.0)
        Wt[j] = val
    return Wt


def _prep_static(mode, spline_kernel, scale_factor, kan_bias, conv_bias):
    if mode == "basis":
        Wt = _prep_basis(spline_kernel, scale_factor)
    else:
        Wt = _prep_fp32(spline_kernel, scale_factor)
    nch = Wt.shape[0]
    wt = np.ascontiguousarray(
        Wt.transpose(1, 0, 2).reshape(128, nch * F), np.float32)

    consts = np.zeros((128, 8), np.float32)
    p = np.arange(128)
    for t in range(3):
        r = 4 * t + p // 32
        consts[:, t] = -(T0 + HGRID * r) / HGRID           # 5.5 - r
        consts[:, 3 + t] = NR - r                           # 11 - r
    consts[:, 6] = (kan_bias.astype(np.float64)
                    + conv_bias.astype(np.float64)).astype(np.float32)
    return wt, consts


def kernel(x, spline_kernel, scale_factor, kan_bias, conv_bias):
    from concourse import bass_utils

    x = np.asarray(x, np.float32)
    spline_kernel = np.asarray(spline_kernel, np.float32)
    scale_factor = np.asarray(scale_factor, np.float32)
    kan_bias = np.asarray(kan_bias, np.float32)
    conv_bias = np.asarray(conv_bias, np.float32)

    key = f"nc_{MODE}"
    if key not in _cache:
        _cache[key] = _build_program(MODE)
    nc = _cache[key]

    wt, consts = _prep_static(MODE, spline_kernel, scale_factor,
                              kan_bias, conv_bias)

    in_maps = []
    for c in range(N_CORES):
        xc = x[c * BPC:(c + 1) * BPC]                      # (4,32,32,32)
        xtc = np.ascontiguousarray(
            xc.transpose(3, 0, 1, 2).reshape(C, BPC * PIX), np.float32
        )
        in_maps.append({"xt": xtc, "wt": wt, "consts": consts})

    res = bass_utils.run_bass_kernel_spmd(
        nc, in_maps, core_ids=list(range(N_CORES)),
        **_cache.get("run_kwargs", {})
    )
    _cache["last_result"] = res

    out = np.empty((B, HO, WO, F), np.float32)
    for c in range(N_CORES):
        yc = res.results[c]["y"]                           # (128, 3600)
        out[c * BPC:(c + 1) * BPC] = (
            yc.reshape(F, BPC, HO, WO).transpose(1, 2, 3, 0)
        )
    return out



# revision 3
# speedup vs baseline: 2.1813x; 1.3848x over previous
"""Conv2D-KAN Trainium2 kernel (8-core data-parallel SPMD), v2.

Formulation
-----------
Per 3x3 patch (N = B*30*30 patches, in_size = 288 = 9 offsets x 32 ch):
    out[n,o] = sum_{i,k} sb[n,i,k] * (spline_kernel*scale)[i,k,o]
             + silu(xf) @ scale_factor + biases
with sb an order-3 B-spline basis (8 funcs) on the uniform grid
t_r = -2.2 + 0.4 r.

Key identities:
 1. Basis values depend only on the underlying *pixel*, so features are
    computed per pixel and the op becomes a 3x3 conv with 128 filters,
    realized as shifted-view matmuls accumulating in PSUM.
 2. For a uniform grid, B_k(x) = b(v), v = (x-t_k)/h - 2, with the
    centered two-term form
        6*b(v) = relu(2-|v|)^3 - 4*relu(1-|v|)^3
    All intermediates are <= 8 (well-conditioned, bf16-safe) and the
    value is *exactly* zero outside the support — so the main matmuls
    can run fully in bf16 (1 cyc/row + fast weight load), with the /6
    folded into the weights.  Equivalent form used on-device:
        Pm = min(|v|,2)-2, Qm = min(|v|,1)-1   (in [-2,0])
        6*b = 4*Qm^3 - Pm^3
 3. The 9 per-offset silu chunks (32 rows each) are packed 4-per-matmul
    by materializing column-shifted SBUF copies of silu(x), cutting the
    matmuls per PSUM bank from 27 to 21.

Per image: 2 basis tiles [128,1024] (4 knots x 32 ch each) built with
ACT(Abs,Square,Silu) + DVE(min-sub, mul, mult-sub) + Pool(mul), then
21 bf16 matmuls per half-image PSUM bank [128 filters, 450 patches].
Each core processes 4 images; output [128, 3600] transposed on host.
"""

import sys

sys.path.insert(0, "/opt/trn_rl_repo")

import numpy as np

N_CORES = 8
B, HH, WW, C = 32, 32, 32, 32
F = 128
KH = KW = 3
HO, WO = HH - KH + 1, WW - KW + 1          # 30, 30
BPC = B // N_CORES                          # images per core = 4
PIX = HH * WW                               # 1024 pixels per image
NPC = BPC * HO * WO                         # 3600 patches per core
BANKN = 450                                 # patches per PSUM bank
HGRID = 0.4
T0 = -2.2
NCHUNK = 21                                 # matmuls per bank
# chunk k -> (kind, arg): 0..8 = B0 offsets, 9..11 = silu s0/s1/s2,
# 12..20 = B1 offsets

_cache = {}


def _build_program():
    import concourse.bacc as bacc
    import concourse.mybir as mybir
    import concourse.tile as tile

    f32 = mybir.dt.float32
    bf16 = mybir.dt.bfloat16
    AF = mybir.ActivationFunctionType
    OP = mybir.AluOpType

    nc = bacc.Bacc("TRN2", target_bir_lowering=False, debug=False)
    xt = nc.dram_tensor("xt", [C, BPC * PIX], f32, kind="ExternalInput").ap()
    wt = nc.dram_tensor("wt", [128, NCHUNK * F], bf16, kind="ExternalInput").ap()
    consts = nc.dram_tensor("consts", [128, 4], f32, kind="ExternalInput").ap()
    y = nc.dram_tensor("y", [F, NPC], f32, kind="ExternalOutput").ap()

    with tile.TileContext(nc) as tc:
        with (
            tc.tile_pool(name="wp", bufs=1) as wp,
            tc.tile_pool(name="cp", bufs=1) as cp,
            tc.tile_pool(name="xp", bufs=2) as xp,
            tc.tile_pool(name="ep", bufs=2) as ep,
            tc.tile_pool(name="bp", bufs=2) as bpool,
            tc.tile_pool(name="op", bufs=1) as op_,
            tc.tile_pool(name="pp", bufs=4, space="PSUM") as pp,
        ):
            ct = cp.tile([128, 4], f32)
            nc.scalar.dma_start(ct[:], consts[:])

            # warm the silu table set (also carries abs/square/identity)
            warm = cp.tile([1, 1], f32, tag="warm")
            nc.scalar.activation(warm[:], ct[:1, :1], AF.Silu)

            # image 0's replica DMAs first, split across two queues
            xr0 = xp.tile([128, PIX], f32, tag="xr")
            eng0 = [nc.sync, nc.scalar, nc.sync, nc.scalar]
            for rep in range(4):
                eng0[rep].dma_start(xr0[32 * rep:32 * rep + 32], xt[:, 0:PIX])

            # weights: chunks 0..11 (B0 + silu) on the scalar queue, the
            # rest on the gpsimd queue in parallel
            wA = wp.tile([128, 12 * F], bf16, tag="wA")
            nc.scalar.dma_start(wA[:], wt[:, :12 * F])
            wB = wp.tile([128, 9 * F], bf16, tag="wB")
            nc.gpsimd.dma_start(wB[:], wt[:, 12 * F:])
            wtiles = [wA[:, i * F:(i + 1) * F] for i in range(12)] + \
                     [wB[:, i * F:(i + 1) * F] for i in range(9)]

            out_t = op_.tile([F, NPC], f32)

            for im in range(BPC):
                sl = slice(im * PIX, (im + 1) * PIX)
                if im == 0:
                    xr = xr0
                else:
                    xr = xp.tile([128, PIX], f32, tag="xr")
                    for rep in range(4):
                        nc.sync.dma_start(
                            xr[32 * rep:32 * rep + 32], xt[:, sl])

                # --- per-pixel features ---------------------------------
                A0 = ep.tile([128, PIX], f32, tag="A0")
                nc.scalar.activation(A0[:], xr[:], AF.Abs,
                                     bias=ct[:, 0:1], scale=1.0 / HGRID)
                A1 = ep.tile([128, PIX], f32, tag="A1")
                nc.scalar.activation(A1[:], xr[:], AF.Abs,
                                     bias=ct[:, 1:2], scale=1.0 / HGRID)

                # silu(x) -> SP0[0:32] (bf16), then shifted SBUF copies
                SP0 = bpool.tile([128, PIX], bf16, tag="SP0")
                SP1 = bpool.tile([128, PIX], bf16, tag="SP1")
                nc.scalar.activation(SP0[0:32], xr[0:32], AF.Silu)
                for off in range(1, 8):
                    di, dj = divmod(off, KW)
                    s = di * WW + dj
                    dst = SP0 if off < 4 else SP1
                    g = off % 4
                    nc.gpsimd.dma_start(
                        dst[32 * g:32 * g + 32, 0:PIX - s],
                        SP0[0:32, s:PIX])

                def halfpipe(Ain, tag, mul_engs):
                    Pm = ep.tile([128, PIX], f32, tag=f"P{tag}")
                    nc.vector.tensor_scalar(
                        Pm[:], Ain[:], 2.0, 2.0, OP.min, OP.subtract)
                    Qm = ep.tile([128, PIX], f32, tag=f"Q{tag}")
                    nc.vector.tensor_scalar(
                        Qm[:], Ain[:], 1.0, 1.0, OP.min, OP.subtract)
                    G = ep.tile([128, PIX], f32, tag=f"G{tag}")
                    nc.scalar.activation(G[:], Pm[:], AF.Square)
                    H = ep.tile([128, PIX], f32, tag=f"H{tag}")
                    nc.scalar.activation(H[:], Qm[:], AF.Square)
                    Cc = ep.tile([128, PIX], f32, tag=f"C{tag}")
                    mul_engs[0].tensor_mul(Cc[:], G[:], Pm[:])
                    Dd = ep.tile([128, PIX], f32, tag=f"D{tag}")
                    mul_engs[1].tensor_mul(Dd[:], H[:], Qm[:])
                    Bt = bpool.tile([128, PIX], bf16, tag=f"B{tag}")
                    # 6*basis = 4*Qm^3 - Pm^3  (the /6 is in the weights)
                    nc.vector.scalar_tensor_tensor(
                        Bt[:], Dd[:], 4.0, Cc[:], OP.mult, OP.subtract)
                    return Bt

                B0 = halfpipe(A0, "0", (nc.vector, nc.gpsimd))
                B1 = halfpipe(A1, "1", (nc.gpsimd, nc.gpsimd))
                B0v = B0[:].rearrange("p (h w) -> p h w", w=WW)
                B1v = B1[:].rearrange("p (h w) -> p h w", w=WW)
                SP0v = SP0[:].rearrange("p (h w) -> p h w", w=WW)
                SP1v = SP1[:].rearrange("p (h w) -> p h w", w=WW)
                SLv = SP0[0:32].rearrange("p (h w) -> p h w", w=WW)

                # --- matmuls --------------------------------------------
                for half in range(2):
                    h0 = half * 15
                    ps = pp.tile([F, BANKN], f32, tag="ps")
                    k = 0

                    def mm(lhsT, rhs, k):
                        nc.tensor.matmul(ps[:], lhsT, rhs,
                                         start=(k == 0), stop=(k == NCHUNK - 1))

                    for off in range(9):
                        di, dj = divmod(off, KW)
                        mm(wtiles[off],
                           B0v[:, h0 + di:h0 + di + 15, dj:dj + WO], k)
                        k += 1
                    mm(wtiles[9], SP0v[:, h0:h0 + 15, 0:WO], k); k += 1
                    mm(wtiles[10], SP1v[:, h0:h0 + 15, 0:WO], k); k += 1
                    mm(wtiles[11][0:32],
                       SLv[:, h0 + 2:h0 + 17, 2:2 + WO], k); k += 1
                    for off in range(9):
                        di, dj = divmod(off, KW)
                        mm(wtiles[12 + off],
                           B1v[:, h0 + di:h0 + di + 15, dj:dj + WO], k)
                        k += 1

                    s = (im * 2 + half) * BANKN
                    nc.scalar.activation(
                        out_t[:, s:s + BANKN], ps[:], AF.Identity,
                        bias=ct[:, 2:3], scale=1.0)
                    nc.sync.dma_start(y[:, s:s + BANKN], out_t[:, s:s + BANKN])

    nc.compile()
    return nc


def _prep_static(spline_kernel, scale_factor, kan_bias, conv_bias):
    import ml_dtypes

    w6 = (spline_kernel.astype(np.float64)
          * scale_factor.astype(np.float64)[:, None, :]) / 6.0
    w6r = w6.reshape(9, 32, 8, F)
    sf = scale_factor.astype(np.float64).reshape(9, 32, F)
    chunks = np.zeros((NCHUNK, 128, F), np.float64)
    for off in range(9):
        chunks[off] = w6r[off, :, 0:4].transpose(1, 0, 2).reshape(128, F)
        chunks[12 + off] = w6r[off, :, 4:8].transpose(1, 0, 2).reshape(128, F)
    for g in range(4):
        chunks[9][g * 32:(g + 1) * 32] = sf[g]
        chunks[10][g * 32:(g + 1) * 32] = sf[4 + g]
    chunks[11][0:32] = sf[8]
    wtc = np.ascontiguousarray(
        chunks.transpose(1, 0, 2).reshape(128, NCHUNK * F))
    wt = wtc.astype(ml_dtypes.bfloat16)

    consts = np.zeros((128, 4), np.float32)
    kl = np.arange(128) // 32
    # v = x/h - k + 3.5  (k = knot index); tile0 k = kl, tile1 k = 4+kl
    consts[:, 0] = 3.5 - kl
    consts[:, 1] = 3.5 - (4 + kl)
    consts[:, 2] = (kan_bias.astype(np.float64)
                    + conv_bias.astype(np.float64)).astype(np.float32)
    return wt, consts


def kernel(x, spline_kernel, scale_factor, kan_bias, conv_bias):
    from concourse import bass_utils

    x = np.asarray(x, np.float32)
    spline_kernel = np.asarray(spline_kernel, np.float32)
    scale_factor = np.asarray(scale_factor, np.float32)
    kan_bias = np.asarray(kan_bias, np.float32)
    conv_bias = np.asarray(conv_bias, np.float32)

    if "nc" not in _cache:
        _cache["nc"] = _build_program()
    nc = _cache["nc"]

    wt, consts = _prep_static(spline_kernel, scale_factor,
                              kan_bias, conv_bias)

    in_maps = []
    for c in range(N_CORES):
        xc = x[c * BPC:(c + 1) * BPC]                      # (4,32,32,32)
        xtc = np.ascontiguousarray(
            xc.transpose(3, 0, 1, 2).reshape(C, BPC * PIX), np.float32
        )
        in_maps.append({"xt": xtc, "wt": wt, "consts": consts})

    res = bass_utils.run_bass_kernel_spmd(
        nc, in_maps, core_ids=list(range(N_CORES)),
        **_cache.get("run_kwargs", {})
    )
    _cache["last_result"] = res

    out = np.empty((B, HO, WO, F), np.float32)
    for c in range(N_CORES):
        yc = res.results[c]["y"]                           # (128, 3600)
        out[c * BPC:(c + 1) * BPC] = (
            yc.reshape(F, BPC, HO, WO).transpose(1, 2, 3, 0)
        )
    return out


# revision 4
# speedup vs baseline: 2.2413x; 1.0275x over previous
"""Conv2D-KAN Trainium2 kernel (8-core data-parallel SPMD), v2.

Formulation
-----------
Per 3x3 patch (N = B*30*30 patches, in_size = 288 = 9 offsets x 32 ch):
    out[n,o] = sum_{i,k} sb[n,i,k] * (spline_kernel*scale)[i,k,o]
             + silu(xf) @ scale_factor + biases
with sb an order-3 B-spline basis (8 funcs) on the uniform grid
t_r = -2.2 + 0.4 r.

Key identities:
 1. Basis values depend only on the underlying *pixel*, so features are
    computed per pixel and the op becomes a 3x3 conv with 128 filters,
    realized as shifted-view matmuls accumulating in PSUM.
 2. For a uniform grid, B_k(x) = b(v), v = (x-t_k)/h - 2, with the
    centered two-term form
        6*b(v) = relu(2-|v|)^3 - 4*relu(1-|v|)^3
    All intermediates are <= 8 (well-conditioned, bf16-safe) and the
    value is *exactly* zero outside the support — so the main matmuls
    can run fully in bf16 (1 cyc/row + fast weight load), with the /6
    folded into the weights.  Equivalent form used on-device:
        Pm = min(|v|,2)-2, Qm = min(|v|,1)-1   (in [-2,0])
        6*b = 4*Qm^3 - Pm^3
 3. The 9 per-offset silu chunks (32 rows each) are packed 4-per-matmul
    by materializing column-shifted SBUF copies of silu(x), cutting the
    matmuls per PSUM bank from 27 to 21.

Per image: 2 basis tiles [128,1024] (4 knots x 32 ch each) built with
ACT(Abs,Square,Silu) + DVE(min-sub, mul, mult-sub) + Pool(mul), then
21 bf16 matmuls per half-image PSUM bank [128 filters, 450 patches].
Each core processes 4 images; output [128, 3600] transposed on host.
"""

import sys

sys.path.insert(0, "/opt/trn_rl_repo")

import numpy as np

N_CORES = 8
B, HH, WW, C = 32, 32, 32, 32
F = 128
KH = KW = 3
HO, WO = HH - KH + 1, WW - KW + 1          # 30, 30
BPC = B // N_CORES                          # images per core = 4
PIX = HH * WW                               # 1024 pixels per image
NPC = BPC * HO * WO                         # 3600 patches per core
BANKN = 450                                 # patches per PSUM bank
HGRID = 0.4
T0 = -2.2
NCHUNK = 21                                 # matmuls per bank
# chunk k -> (kind, arg): 0..8 = B0 offsets, 9..11 = silu s0/s1/s2,
# 12..20 = B1 offsets

_cache = {}


def _build_program():
    import concourse.bacc as bacc
    import concourse.mybir as mybir
    import concourse.tile as tile

    f32 = mybir.dt.float32
    bf16 = mybir.dt.bfloat16
    AF = mybir.ActivationFunctionType
    OP = mybir.AluOpType

    nc = bacc.Bacc("TRN2", target_bir_lowering=False, debug=False)
    xt = nc.dram_tensor("xt", [C, BPC * PIX], f32, kind="ExternalInput").ap()
    wt = nc.dram_tensor("wt", [128, NCHUNK * F], bf16, kind="ExternalInput").ap()
    consts = nc.dram_tensor("consts", [128, 4], f32, kind="ExternalInput").ap()
    y = nc.dram_tensor("y", [F, NPC], f32, kind="ExternalOutput").ap()

    with tile.TileContext(nc) as tc:
        with (
            tc.tile_pool(name="wp", bufs=1) as wp,
            tc.tile_pool(name="cp", bufs=1) as cp,
            tc.tile_pool(name="xp", bufs=2) as xp,
            tc.tile_pool(name="ep", bufs=2) as ep,
            tc.tile_pool(name="bp", bufs=2) as bpool,
            tc.tile_pool(name="op", bufs=1) as op_,
            tc.tile_pool(name="pp", bufs=4, space="PSUM") as pp,
        ):
            ct = cp.tile([128, 4], f32)
            nc.scalar.dma_start(ct[:], consts[:])

            # warm the silu table set (also carries abs/square/identity)
            warm = cp.tile([1, 1], f32, tag="warm")
            nc.scalar.activation(warm[:], ct[:1, :1], AF.Silu)

            # image 0's replica DMAs first, split across two queues
            xr0 = xp.tile([128, PIX], f32, tag="xr")
            eng0 = [nc.sync, nc.scalar, nc.sync, nc.scalar]
            for rep in range(4):
                eng0[rep].dma_start(xr0[32 * rep:32 * rep + 32], xt[:, 0:PIX])

            # weights: chunks 0..11 (B0 + silu) on the scalar queue, the
            # rest on the gpsimd queue in parallel
            wA = wp.tile([128, 12 * F], bf16, tag="wA")
            nc.scalar.dma_start(wA[:], wt[:, :12 * F])
            wB = wp.tile([128, 9 * F], bf16, tag="wB")
            nc.gpsimd.dma_start(wB[:], wt[:, 12 * F:])
            wtiles = [wA[:, i * F:(i + 1) * F] for i in range(12)] + \
                     [wB[:, i * F:(i + 1) * F] for i in range(9)]

            out_t = op_.tile([F, NPC], f32)

            for im in range(BPC):
                sl = slice(im * PIX, (im + 1) * PIX)
                if im == 0:
                    xr = xr0
                else:
                    xr = xp.tile([128, PIX], f32, tag="xr")
                    for rep in range(4):
                        nc.sync.dma_start(
                            xr[32 * rep:32 * rep + 32], xt[:, sl])

                # --- per-pixel features (bf16: 2x DVE throughput) -------
                A0 = ep.tile([128, PIX], bf16, tag="A0")
                nc.scalar.activation(A0[:], xr[:], AF.Abs,
                                     bias=ct[:, 0:1], scale=1.0 / HGRID)
                A1 = ep.tile([128, PIX], bf16, tag="A1")
                nc.scalar.activation(A1[:], xr[:], AF.Abs,
                                     bias=ct[:, 1:2], scale=1.0 / HGRID)

                # silu(x) -> SP0[0:32] (bf16), then shifted SBUF copies
                SP0 = bpool.tile([128, PIX], bf16, tag="SP0")
                SP1 = bpool.tile([128, PIX], bf16, tag="SP1")
                nc.scalar.activation(SP0[0:32], xr[0:32], AF.Silu)
                for off in range(1, 8):
                    di, dj = divmod(off, KW)
                    s = di * WW + dj
                    dst = SP0 if off < 4 else SP1
                    g = off % 4
                    nc.sync.dma_start(
                        dst[32 * g:32 * g + 32, 0:PIX - s],
                        SP0[0:32, s:PIX])

                def halfpipe(Ain, tag, mul_engs):
                    Pm = ep.tile([128, PIX], bf16, tag=f"P{tag}")
                    nc.vector.tensor_scalar(
                        Pm[:], Ain[:], 2.0, 2.0, OP.min, OP.subtract)
                    Qm = ep.tile([128, PIX], bf16, tag=f"Q{tag}")
                    nc.vector.tensor_scalar(
                        Qm[:], Ain[:], 1.0, 1.0, OP.min, OP.subtract)
                    G = ep.tile([128, PIX], bf16, tag=f"G{tag}")
                    nc.scalar.activation(G[:], Pm[:], AF.Square)
                    H = ep.tile([128, PIX], bf16, tag=f"H{tag}")
                    nc.scalar.activation(H[:], Qm[:], AF.Square)
                    Cc = ep.tile([128, PIX], bf16, tag=f"C{tag}")
                    mul_engs[0].tensor_mul(Cc[:], G[:], Pm[:])
                    Dd = ep.tile([128, PIX], bf16, tag=f"D{tag}")
                    mul_engs[1].tensor_mul(Dd[:], H[:], Qm[:])
                    Bt = bpool.tile([128, PIX], bf16, tag=f"B{tag}")
                    # 6*basis = 4*Qm^3 - Pm^3  (the /6 is in the weights)
                    nc.vector.scalar_tensor_tensor(
                        Bt[:], Dd[:], 4.0, Cc[:], OP.mult, OP.subtract)
                    return Bt

                B0 = halfpipe(A0, "0", (nc.vector, nc.gpsimd))
                B1 = halfpipe(A1, "1", (nc.gpsimd, nc.gpsimd))
                B0v = B0[:].rearrange("p (h w) -> p h w", w=WW)
                B1v = B1[:].rearrange("p (h w) -> p h w", w=WW)
                SP0v = SP0[:].rearrange("p (h w) -> p h w", w=WW)
                SP1v = SP1[:].rearrange("p (h w) -> p h w", w=WW)
                SLv = SP0[0:32].rearrange("p (h w) -> p h w", w=WW)

                # --- matmuls --------------------------------------------
                for half in range(2):
                    h0 = half * 15
                    ps = pp.tile([F, BANKN], f32, tag="ps")
                    k = 0

                    def mm(lhsT, rhs, k):
                        nc.tensor.matmul(ps[:], lhsT, rhs,
                                         start=(k == 0), stop=(k == NCHUNK - 1))

                    for off in range(9):
                        di, dj = divmod(off, KW)
                        mm(wtiles[off],
                           B0v[:, h0 + di:h0 + di + 15, dj:dj + WO], k)
                        k += 1
                    mm(wtiles[9], SP0v[:, h0:h0 + 15, 0:WO], k); k += 1
                    mm(wtiles[10], SP1v[:, h0:h0 + 15, 0:WO], k); k += 1
                    mm(wtiles[11][0:32],
                       SLv[:, h0 + 2:h0 + 17, 2:2 + WO], k); k += 1
                    for off in range(9):
                        di, dj = divmod(off, KW)
                        mm(wtiles[12 + off],
                           B1v[:, h0 + di:h0 + di + 15, dj:dj + WO], k)
                        k += 1

                    s = (im * 2 + half) * BANKN
                    nc.scalar.activation(
                        out_t[:, s:s + BANKN], ps[:], AF.Identity,
                        bias=ct[:, 2:3], scale=1.0)
                    nc.sync.dma_start(y[:, s:s + BANKN], out_t[:, s:s + BANKN])

    nc.compile()
    return nc


def _prep_static(spline_kernel, scale_factor, kan_bias, conv_bias):
    import ml_dtypes

    w6 = (spline_kernel.astype(np.float64)
          * scale_factor.astype(np.float64)[:, None, :]) / 6.0
    w6r = w6.reshape(9, 32, 8, F)
    sf = scale_factor.astype(np.float64).reshape(9, 32, F)
    chunks = np.zeros((NCHUNK, 128, F), np.float64)
    for off in range(9):
        chunks[off] = w6r[off, :, 0:4].transpose(1, 0, 2).reshape(128, F)
        chunks[12 + off] = w6r[off, :, 4:8].transpose(1, 0, 2).reshape(128, F)
    for g in range(4):
        chunks[9][g * 32:(g + 1) * 32] = sf[g]
        chunks[10][g * 32:(g + 1) * 32] = sf[4 + g]
    chunks[11][0:32] = sf[8]
    wtc = np.ascontiguousarray(
        chunks.transpose(1, 0, 2).reshape(128, NCHUNK * F))
    wt = wtc.astype(ml_dtypes.bfloat16)

    consts = np.zeros((128, 4), np.float32)
    kl = np.arange(128) // 32
    # v = x/h - k + 3.5  (k = knot index); tile0 k = kl, tile1 k = 4+kl
    consts[:, 0] = 3.5 - kl
    consts[:, 1] = 3.5 - (4 + kl)
    consts[:, 2] = (kan_bias.astype(np.float64)
                    + conv_bias.astype(np.float64)).astype(np.float32)
    return wt, consts


def kernel(x, spline_kernel, scale_factor, kan_bias, conv_bias):
    from concourse import bass_utils

    x = np.asarray(x, np.float32)
    spline_kernel = np.asarray(spline_kernel, np.float32)
    scale_factor = np.asarray(scale_factor, np.float32)
    kan_bias = np.asarray(kan_bias, np.float32)
    conv_bias = np.asarray(conv_bias, np.float32)

    if "nc" not in _cache:
        _cache["nc"] = _build_program()
    nc = _cache["nc"]

    wt, consts = _prep_static(spline_kernel, scale_factor,
                              kan_bias, conv_bias)

    in_maps = []
    for c in range(N_CORES):
        xc = x[c * BPC:(c + 1) * BPC]                      # (4,32,32,32)
        xtc = np.ascontiguousarray(
            xc.transpose(3, 0, 1, 2).reshape(C, BPC * PIX), np.float32
        )
        in_maps.append({"xt": xtc, "wt": wt, "consts": consts})

    res = bass_utils.run_bass_kernel_spmd(
        nc, in_maps, core_ids=list(range(N_CORES)),
        **_cache.get("run_kwargs", {})
    )
    _cache["last_result"] = res

    out = np.empty((B, HO, WO, F), np.float32)
    for c in range(N_CORES):
        yc = res.results[c]["y"]                           # (128, 3600)
        out[c * BPC:(c + 1) * BPC] = (
            yc.reshape(F, BPC, HO, WO).transpose(1, 2, 3, 0)
        )
    return out


# revision 5
# speedup vs baseline: 2.2833x; 1.0187x over previous
"""Conv2D-KAN Trainium2 kernel (8-core data-parallel SPMD), v2.

Formulation
-----------
Per 3x3 patch (N = B*30*30 patches, in_size = 288 = 9 offsets x 32 ch):
    out[n,o] = sum_{i,k} sb[n,i,k] * (spline_kernel*scale)[i,k,o]
             + silu(xf) @ scale_factor + biases
with sb an order-3 B-spline basis (8 funcs) on the uniform grid
t_r = -2.2 + 0.4 r.

Key identities:
 1. Basis values depend only on the underlying *pixel*, so features are
    computed per pixel and the op becomes a 3x3 conv with 128 filters,
    realized as shifted-view matmuls accumulating in PSUM.
 2. For a uniform grid, B_k(x) = b(v), v = (x-t_k)/h - 2, with the
    centered two-term form
        6*b(v) = relu(2-|v|)^3 - 4*relu(1-|v|)^3
    All intermediates are <= 8 (well-conditioned, bf16-safe) and the
    value is *exactly* zero outside the support — so the main matmuls
    can run fully in bf16 (1 cyc/row + fast weight load), with the /6
    folded into the weights.  Equivalent form used on-device:
        Pm = min(|v|,2)-2, Qm = min(|v|,1)-1   (in [-2,0])
        6*b = 4*Qm^3 - Pm^3
 3. The 9 per-offset silu chunks (32 rows each) are packed 4-per-matmul
    by materializing column-shifted SBUF copies of silu(x), cutting the
    matmuls per PSUM bank from 27 to 21.

Per image: 2 basis tiles [128,1024] (4 knots x 32 ch each) built with
ACT(Abs,Square,Silu) + DVE(min-sub, mul, mult-sub) + Pool(mul), then
21 bf16 matmuls per half-image PSUM bank [128 filters, 450 patches].
Each core processes 4 images; output [128, 3600] transposed on host.
"""

import sys

sys.path.insert(0, "/opt/trn_rl_repo")

import numpy as np

N_CORES = 8
B, HH, WW, C = 32, 32, 32, 32
F = 128
KH = KW = 3
HO, WO = HH - KH + 1, WW - KW + 1          # 30, 30
BPC = B // N_CORES                          # images per core = 4
PIX = HH * WW                               # 1024 pixels per image
NPC = BPC * HO * WO                         # 3600 patches per core
BANKN = 450                                 # patches per PSUM bank
HGRID = 0.4
T0 = -2.2
NCHUNK = 21                                 # matmuls per bank
# chunk k -> (kind, arg): 0..8 = B0 offsets, 9..11 = silu s0/s1/s2,
# 12..20 = B1 offsets

_cache = {}


def _build_program():
    import concourse.bacc as bacc
    import concourse.mybir as mybir
    import concourse.tile as tile

    f32 = mybir.dt.float32
    bf16 = mybir.dt.bfloat16
    AF = mybir.ActivationFunctionType
    OP = mybir.AluOpType

    nc = bacc.Bacc("TRN2", target_bir_lowering=False, debug=False)
    xt = nc.dram_tensor("xt", [C, BPC * PIX], f32, kind="ExternalInput").ap()
    wt = nc.dram_tensor("wt", [128, NCHUNK * F], bf16, kind="ExternalInput").ap()
    consts = nc.dram_tensor("consts", [128, 4], f32, kind="ExternalInput").ap()
    y = nc.dram_tensor("y", [F, NPC], f32, kind="ExternalOutput").ap()

    with tile.TileContext(nc) as tc:
        with (
            tc.tile_pool(name="wp", bufs=1) as wp,
            tc.tile_pool(name="cp", bufs=1) as cp,
            tc.tile_pool(name="xp", bufs=2) as xp,
            tc.tile_pool(name="ep", bufs=2) as ep,
            tc.tile_pool(name="bp", bufs=2) as bpool,
            tc.tile_pool(name="op", bufs=1) as op_,
            tc.tile_pool(name="pp", bufs=4, space="PSUM") as pp,
        ):
            ct = cp.tile([128, 4], f32)
            nc.scalar.dma_start(ct[:], consts[:])

            # warm the silu table set (also carries abs/square/identity)
            warm = cp.tile([1, 1], f32, tag="warm")
            nc.scalar.activation(warm[:], ct[:1, :1], AF.Silu)

            # image 0's replica DMAs first, split across two queues
            xr0 = xp.tile([128, PIX], f32, tag="xr")
            eng0 = [nc.sync, nc.scalar, nc.sync, nc.scalar]
            for rep in range(4):
                eng0[rep].dma_start(xr0[32 * rep:32 * rep + 32], xt[:, 0:PIX])

            # weights: chunks 0..11 (B0 + silu) on the scalar queue, the
            # rest on the gpsimd queue in parallel
            wA = wp.tile([128, 12 * F], bf16, tag="wA")
            nc.scalar.dma_start(wA[:], wt[:, :12 * F])
            wB = wp.tile([128, 9 * F], bf16, tag="wB")
            nc.gpsimd.dma_start(wB[:], wt[:, 12 * F:])
            wtiles = [wA[:, i * F:(i + 1) * F] for i in range(12)] + \
                     [wB[:, i * F:(i + 1) * F] for i in range(9)]

            out_t = op_.tile([F, NPC], f32)
            pending = []

            for im in range(BPC):
                sl = slice(im * PIX, (im + 1) * PIX)
                if im == 0:
                    xr = xr0
                else:
                    xr = xp.tile([128, PIX], f32, tag="xr")
                    for rep in range(4):
                        nc.sync.dma_start(
                            xr[32 * rep:32 * rep + 32], xt[:, sl])

                # --- per-pixel features (bf16: 2x DVE throughput) -------
                A0 = ep.tile([128, PIX], bf16, tag="A0")
                nc.scalar.activation(A0[:], xr[:], AF.Abs,
                                     bias=ct[:, 0:1], scale=1.0 / HGRID)
                A1 = ep.tile([128, PIX], bf16, tag="A1")
                nc.scalar.activation(A1[:], xr[:], AF.Abs,
                                     bias=ct[:, 1:2], scale=1.0 / HGRID)

                # silu(x) -> SP0[0:32] (bf16), then shifted SBUF copies
                SP0 = bpool.tile([128, PIX], bf16, tag="SP0")
                SP1 = bpool.tile([128, PIX], bf16, tag="SP1")
                nc.scalar.activation(SP0[0:32], xr[0:32], AF.Silu)
                for off in range(1, 8):
                    di, dj = divmod(off, KW)
                    s = di * WW + dj
                    dst = SP0 if off < 4 else SP1
                    g = off % 4
                    nc.sync.dma_start(
                        dst[32 * g:32 * g + 32, 0:PIX - s],
                        SP0[0:32, s:PIX])

                def halfpipe(Ain, tag, mul_engs):
                    Pm = ep.tile([128, PIX], bf16, tag=f"P{tag}")
                    nc.vector.tensor_scalar(
                        Pm[:], Ain[:], 2.0, 2.0, OP.min, OP.subtract)
                    Qm = ep.tile([128, PIX], bf16, tag=f"Q{tag}")
                    nc.vector.tensor_scalar(
                        Qm[:], Ain[:], 1.0, 1.0, OP.min, OP.subtract)
                    G = ep.tile([128, PIX], bf16, tag=f"G{tag}")
                    nc.scalar.activation(G[:], Pm[:], AF.Square)
                    H = ep.tile([128, PIX], bf16, tag=f"H{tag}")
                    nc.scalar.activation(H[:], Qm[:], AF.Square)
                    Cc = ep.tile([128, PIX], bf16, tag=f"C{tag}")
                    mul_engs[0].tensor_mul(Cc[:], G[:], Pm[:])
                    Dd = ep.tile([128, PIX], bf16, tag=f"D{tag}")
                    mul_engs[1].tensor_mul(Dd[:], H[:], Qm[:])
                    Bt = bpool.tile([128, PIX], bf16, tag=f"B{tag}")
                    # 6*basis = 4*Qm^3 - Pm^3  (the /6 is in the weights)
                    nc.vector.scalar_tensor_tensor(
                        Bt[:], Dd[:], 4.0, Cc[:], OP.mult, OP.subtract)
                    return Bt

                B0 = halfpipe(A0, "0", (nc.vector, nc.gpsimd))
                B1 = halfpipe(A1, "1", (nc.gpsimd, nc.gpsimd))
                B0v = B0[:].rearrange("p (h w) -> p h w", w=WW)
                B1v = B1[:].rearrange("p (h w) -> p h w", w=WW)
                SP0v = SP0[:].rearrange("p (h w) -> p h w", w=WW)
                SP1v = SP1[:].rearrange("p (h w) -> p h w", w=WW)
                SLv = SP0[0:32].rearrange("p (h w) -> p h w", w=WW)

                # --- matmuls --------------------------------------------
                for half in range(2):
                    h0 = half * 15
                    ps = pp.tile([F, BANKN], f32, tag="ps")
                    k = 0

                    def mm(lhsT, rhs, k):
                        nc.tensor.matmul(ps[:], lhsT, rhs,
                                         start=(k == 0), stop=(k == NCHUNK - 1))

                    for off in range(9):
                        di, dj = divmod(off, KW)
                        mm(wtiles[off],
                           B0v[:, h0 + di:h0 + di + 15, dj:dj + WO], k)
                        k += 1
                    mm(wtiles[9], SP0v[:, h0:h0 + 15, 0:WO], k); k += 1
                    mm(wtiles[10], SP1v[:, h0:h0 + 15, 0:WO], k); k += 1
                    mm(wtiles[11][0:32],
                       SLv[:, h0 + 2:h0 + 17, 2:2 + WO], k); k += 1
                    for off in range(9):
                        di, dj = divmod(off, KW)
                        mm(wtiles[12 + off],
                           B1v[:, h0 + di:h0 + di + 15, dj:dj + WO], k)
                        k += 1
                    pending.append(((im * 2 + half) * BANKN, ps))

                # drain the PREVIOUS image's PSUM banks only after this
                # image's feature + matmul issue, so ACT/DVE run ahead of
                # the PE instead of stalling on its PSUM completion.
                while len(pending) > 2:
                    s, ps = pending.pop(0)
                    nc.scalar.activation(
                        out_t[:, s:s + BANKN], ps[:], AF.Identity,
                        bias=ct[:, 2:3], scale=1.0)
                    nc.sync.dma_start(y[:, s:s + BANKN], out_t[:, s:s + BANKN])

            while pending:
                s, ps = pending.pop(0)
                nc.scalar.activation(
                    out_t[:, s:s + BANKN], ps[:], AF.Identity,
                    bias=ct[:, 2:3], scale=1.0)
                nc.sync.dma_start(y[:, s:s + BANKN], out_t[:, s:s + BANKN])

    nc.compile()
    return nc


def _prep_static(spline_kernel, scale_factor, kan_bias, conv_bias):
    import ml_dtypes

    w6 = (spline_kernel.astype(np.float64)
          * scale_factor.astype(np.float64)[:, None, :]) / 6.0
    w6r = w6.reshape(9, 32, 8, F)
    sf = scale_factor.astype(np.float64).reshape(9, 32, F)
    chunks = np.zeros((NCHUNK, 128, F), np.float64)
    for off in range(9):
        chunks[off] = w6r[off, :, 0:4].transpose(1, 0, 2).reshape(128, F)
        chunks[12 + off] = w6r[off, :, 4:8].transpose(1, 0, 2).reshape(128, F)
    for g in range(4):
        chunks[9][g * 32:(g + 1) * 32] = sf[g]
        chunks[10][g * 32:(g + 1) * 32] = sf[4 + g]
    chunks[11][0:32] = sf[8]
    wtc = np.ascontiguousarray(
        chunks.transpose(1, 0, 2).reshape(128, NCHUNK * F))
    wt = wtc.astype(ml_dtypes.bfloat16)

    consts = np.zeros((128, 4), np.float32)
    kl = np.arange(128) // 32
    # v = x/h - k + 3.5  (k = knot index); tile0 k = kl, tile1 k = 4+kl
    consts[:, 0] = 3.5 - kl
    consts[:, 1] = 3.5 - (4 + kl)
    consts[:, 2] = (kan_bias.astype(np.float64)
                    + conv_bias.astype(np.float64)).astype(np.float32)
    return wt, consts


def kernel(x, spline_kernel, scale_factor, kan_bias, conv_bias):
    from concourse import bass_utils

    x = np.asarray(x, np.float32)
    spline_kernel = np.asarray(spline_kernel, np.float32)
    scale_factor = np.asarray(scale_factor, np.float32)
    kan_bias = np.asarray(kan_bias, np.float32)
    conv_bias = np.asarray(conv_bias, np.float32)

    if "nc" not in _cache:
        _cache["nc"] = _build_program()
    nc = _cache["nc"]

    wt, consts = _prep_static(spline_kernel, scale_factor,
                              kan_bias, conv_bias)

    in_maps = []
    for c in range(N_CORES):
        xc = x[c * BPC:(c + 1) * BPC]                      # (4,32,32,32)
        xtc = np.ascontiguousarray(
            xc.transpose(3, 0, 1, 2).reshape(C, BPC * PIX), np.float32
        )
        in_maps.append({"xt": xtc, "wt": wt, "consts": consts})

    res = bass_utils.run_bass_kernel_spmd(
        nc, in_maps, core_ids=list(range(N_CORES)),
        **_cache.get("run_kwargs", {})
    )
    _cache["last_result"] = res

    out = np.empty((B, HO, WO, F), np.float32)
    for c in range(N_CORES):
        yc = res.results[c]["y"]                           # (128, 3600)
        out[c * BPC:(c + 1) * BPC] = (
            yc.reshape(F, BPC, HO, WO).transpose(1, 2, 3, 0)
        )
    return out


# revision 8
# speedup vs baseline: 2.5552x; 1.1191x over previous
"""Conv2D-KAN Trainium2 kernel (8-core data-parallel SPMD), v2.

Formulation
-----------
Per 3x3 patch (N = B*30*30 patches, in_size = 288 = 9 offsets x 32 ch):
    out[n,o] = sum_{i,k} sb[n,i,k] * (spline_kernel*scale)[i,k,o]
             + silu(xf) @ scale_factor + biases
with sb an order-3 B-spline basis (8 funcs) on the uniform grid
t_r = -2.2 + 0.4 r.

Key identities:
 1. Basis values depend only on the underlying *pixel*, so features are
    computed per pixel and the op becomes a 3x3 conv with 128 filters,
    realized as shifted-view matmuls accumulating in PSUM.
 2. For a uniform grid, B_k(x) = b(v), v = (x-t_k)/h - 2, with the
    centered two-term form
        6*b(v) = relu(2-|v|)^3 - 4*relu(1-|v|)^3
    All intermediates are <= 8 (well-conditioned, bf16-safe) and the
    value is *exactly* zero outside the support — so the main matmuls
    can run fully in bf16 (1 cyc/row + fast weight load), with the /6
    folded into the weights.  Equivalent form used on-device:
        Pm = min(|v|,2)-2, Qm = min(|v|,1)-1   (in [-2,0])
        6*b = 4*Qm^3 - Pm^3
 3. The 9 per-offset silu chunks (32 rows each) are packed 4-per-matmul
    by materializing column-shifted SBUF copies of silu(x), cutting the
    matmuls per PSUM bank from 27 to 21.

Per image: 2 basis tiles [128,1024] (4 knots x 32 ch each) built with
ACT(Abs,Square,Silu) + DVE(min-sub, mul, mult-sub) + Pool(mul), then
21 bf16 matmuls per half-image PSUM bank [128 filters, 450 patches].
Each core processes 4 images; output [128, 3600] transposed on host.
"""

import sys

sys.path.insert(0, "/opt/trn_rl_repo")

import numpy as np

N_CORES = 8
B, HH, WW, C = 32, 32, 32, 32
F = 128
KH = KW = 3
HO, WO = HH - KH + 1, WW - KW + 1          # 30, 30
BPC = B // N_CORES                          # images per core = 4
PIX = HH * WW                               # 1024 pixels per image
NPC = BPC * HO * WO                         # 3600 patches per core
BANKN = 450                                 # patches per PSUM bank
HGRID = 0.4
T0 = -2.2
NCHUNK = 21                                 # matmuls per bank
# chunk k -> (kind, arg): 0..8 = B0 offsets, 9..11 = silu s0/s1/s2,
# 12..20 = B1 offsets

_cache = {}


def _build_program():
    import concourse.bacc as bacc
    import concourse.mybir as mybir
    import concourse.tile as tile

    f32 = mybir.dt.float32
    bf16 = mybir.dt.bfloat16
    AF = mybir.ActivationFunctionType
    OP = mybir.AluOpType

    nc = bacc.Bacc("TRN2", target_bir_lowering=False, debug=False)
    xt = nc.dram_tensor("xt", [C, BPC * PIX], f32, kind="ExternalInput").ap()
    wt = nc.dram_tensor("wt", [128, NCHUNK * F], bf16, kind="ExternalInput").ap()
    consts = nc.dram_tensor("consts", [128, 4], f32, kind="ExternalInput").ap()
    y = nc.dram_tensor("y", [F, NPC], f32, kind="ExternalOutput").ap()

    with tile.TileContext(nc) as tc:
        with (
            tc.tile_pool(name="wp", bufs=1) as wp,
            tc.tile_pool(name="cp", bufs=1) as cp,
            tc.tile_pool(name="xp", bufs=2) as xp,
            tc.tile_pool(name="ep", bufs=2) as ep,
            tc.tile_pool(name="bp", bufs=2) as bpool,
            tc.tile_pool(name="op", bufs=1) as op_,
            tc.tile_pool(name="pp", bufs=4, space="PSUM") as pp,
        ):
            ct = cp.tile([128, 4], f32)
            nc.scalar.dma_start(ct[:], consts[:])

            # warm the silu table set (also carries abs/square/identity)
            warm = cp.tile([1, 1], f32, tag="warm")
            nc.scalar.activation(warm[:], ct[:1, :1], AF.Silu)

            # image 0's replica DMAs first, split across two queues
            xr0 = xp.tile([128, PIX], f32, tag="xr")
            eng0 = [nc.sync, nc.scalar, nc.sync, nc.scalar]
            for rep in range(4):
                eng0[rep].dma_start(xr0[32 * rep:32 * rep + 32], xt[:, 0:PIX])

            # weights: chunks 0..11 (B0 + silu) on the scalar queue, the
            # rest on the gpsimd queue in parallel
            wA = wp.tile([128, 12 * F], bf16, tag="wA")
            nc.scalar.dma_start(wA[:], wt[:, :12 * F])
            wB = wp.tile([128, 9 * F], bf16, tag="wB")
            nc.gpsimd.dma_start(wB[:], wt[:, 12 * F:])
            wtiles = [wA[:, i * F:(i + 1) * F] for i in range(12)] + \
                     [wB[:, i * F:(i + 1) * F] for i in range(9)]

            out_t = op_.tile([F, NPC], f32)
            pending = []

            for im in range(BPC):
                sl = slice(im * PIX, (im + 1) * PIX)
                if im == 0:
                    xr = xr0
                else:
                    xr = xp.tile([128, PIX], f32, tag="xr")
                    for rep in range(4):
                        nc.sync.dma_start(
                            xr[32 * rep:32 * rep + 32], xt[:, sl])

                # --- per-pixel features (bf16: 2x DVE throughput) -------
                A0 = ep.tile([128, PIX], bf16, tag="A0")
                nc.scalar.activation(A0[:], xr[:], AF.Abs,
                                     bias=ct[:, 0:1], scale=1.0 / HGRID)
                A1 = ep.tile([128, PIX], bf16, tag="A1")
                nc.scalar.activation(A1[:], xr[:], AF.Abs,
                                     bias=ct[:, 1:2], scale=1.0 / HGRID)

                # silu(x) -> SP0[0:32] (bf16), then shifted SBUF copies
                SP0 = bpool.tile([128, PIX], bf16, tag="SP0")
                SP1 = bpool.tile([128, PIX], bf16, tag="SP1")
                nc.scalar.activation(SP0[0:32], xr[0:32], AF.Silu)
                for off in range(1, 8):
                    di, dj = divmod(off, KW)
                    s = di * WW + dj
                    dst = SP0 if off < 4 else SP1
                    g = off % 4
                    nc.sync.dma_start(
                        dst[32 * g:32 * g + 32, 0:PIX - s],
                        SP0[0:32, s:PIX])

                def halfpipe(Ain, tag, mul_engs):
                    Pm = ep.tile([128, PIX], bf16, tag=f"P{tag}")
                    nc.vector.tensor_scalar(
                        Pm[:], Ain[:], 2.0, 2.0, OP.min, OP.subtract)
                    Qm = ep.tile([128, PIX], bf16, tag=f"Q{tag}")
                    nc.vector.tensor_scalar(
                        Qm[:], Ain[:], 1.0, 1.0, OP.min, OP.subtract)
                    G = ep.tile([128, PIX], bf16, tag=f"G{tag}")
                    nc.scalar.activation(G[:], Pm[:], AF.Square)
                    H = ep.tile([128, PIX], bf16, tag=f"H{tag}")
                    nc.scalar.activation(H[:], Qm[:], AF.Square)
                    Cc = ep.tile([128, PIX], bf16, tag=f"C{tag}")
                    mul_engs[0].tensor_mul(Cc[:], G[:], Pm[:])
                    Dd = ep.tile([128, PIX], bf16, tag=f"D{tag}")
                    mul_engs[1].tensor_mul(Dd[:], H[:], Qm[:])
                    Bt = bpool.tile([128, PIX], bf16, tag=f"B{tag}")
                    # 6*basis = 4*Qm^3 - Pm^3  (the /6 is in the weights)
                    nc.vector.scalar_tensor_tensor(
                        Bt[:], Dd[:], 4.0, Cc[:], OP.mult, OP.subtract)
                    return Bt

                B0 = halfpipe(A0, "0", (nc.vector, nc.gpsimd))
                B1 = halfpipe(A1, "1", (nc.vector, nc.gpsimd))
                B0v = B0[:].rearrange("p (h w) -> p h w", w=WW)
                B1v = B1[:].rearrange("p (h w) -> p h w", w=WW)
                SP0v = SP0[:].rearrange("p (h w) -> p h w", w=WW)
                SP1v = SP1[:].rearrange("p (h w) -> p h w", w=WW)
                SLv = SP0[0:32].rearrange("p (h w) -> p h w", w=WW)

                # --- matmuls: both banks' B0+silu groups first, so the
                # late-arriving B1 tile is only needed ~5us into the image
                pss = [pp.tile([F, BANKN], f32, tag="ps", name=f"ps{im}_{h_}") for h_ in range(2)]
                for half in range(2):
                    h0 = half * 15
                    ps = pss[half]
                    for k, off in enumerate(range(9)):
                        di, dj = divmod(off, KW)
                        nc.tensor.matmul(
                            ps[:], wtiles[off],
                            B0v[:, h0 + di:h0 + di + 15, dj:dj + WO],
                            start=(k == 0), stop=False)
                    nc.tensor.matmul(ps[:], wtiles[9],
                                     SP0v[:, h0:h0 + 15, 0:WO],
                                     start=False, stop=False)
                    nc.tensor.matmul(ps[:], wtiles[10],
                                     SP1v[:, h0:h0 + 15, 0:WO],
                                     start=False, stop=False)
                    nc.tensor.matmul(ps[:], wtiles[11][0:32],
                                     SLv[:, h0 + 2:h0 + 17, 2:2 + WO],
                                     start=False, stop=False)
                for half in range(2):
                    h0 = half * 15
                    ps = pss[half]
                    for k, off in enumerate(range(9)):
                        di, dj = divmod(off, KW)
                        nc.tensor.matmul(
                            ps[:], wtiles[12 + off],
                            B1v[:, h0 + di:h0 + di + 15, dj:dj + WO],
                            start=False, stop=(k == 8))
                    pending.append(((im * 2 + half) * BANKN, ps))

                # drain the PREVIOUS image's PSUM banks only after this
                # image's feature + matmul issue, so ACT/DVE run ahead of
                # the PE instead of stalling on its PSUM completion.
                while len(pending) > 2:
                    s, ps = pending.pop(0)
                    nc.scalar.activation(
                        out_t[:, s:s + BANKN], ps[:], AF.Identity,
                        bias=ct[:, 2:3], scale=1.0)
                    nc.sync.dma_start(y[:, s:s + BANKN], out_t[:, s:s + BANKN])

            while pending:
                s, ps = pending.pop(0)
                nc.scalar.activation(
                    out_t[:, s:s + BANKN], ps[:], AF.Identity,
                    bias=ct[:, 2:3], scale=1.0)
                nc.sync.dma_start(y[:, s:s + BANKN], out_t[:, s:s + BANKN])

    nc.compile()
    return nc


def _prep_static(spline_kernel, scale_factor, kan_bias, conv_bias):
    import ml_dtypes

    w6 = (spline_kernel.astype(np.float64)
          * scale_factor.astype(np.float64)[:, None, :]) / 6.0
    w6r = w6.reshape(9, 32, 8, F)
    sf = scale_factor.astype(np.float64).reshape(9, 32, F)
    chunks = np.zeros((NCHUNK, 128, F), np.float64)
    for off in range(9):
        chunks[off] = w6r[off, :, 0:4].transpose(1, 0, 2).reshape(128, F)
        chunks[12 + off] = w6r[off, :, 4:8].transpose(1, 0, 2).reshape(128, F)
    for g in range(4):
        chunks[9][g * 32:(g + 1) * 32] = sf[g]
        chunks[10][g * 32:(g + 1) * 32] = sf[4 + g]
    chunks[11][0:32] = sf[8]
    wtc = np.ascontiguousarray(
        chunks.transpose(1, 0, 2).reshape(128, NCHUNK * F))
    wt = wtc.astype(ml_dtypes.bfloat16)

    consts = np.zeros((128, 4), np.float32)
    kl = np.arange(128) // 32
    # v = x/h - k + 3.5  (k = knot index); tile0 k = kl, tile1 k = 4+kl
    consts[:, 0] = 3.5 - kl
    consts[:, 1] = 3.5 - (4 + kl)
    consts[:, 2] = (kan_bias.astype(np.float64)
                    + conv_bias.astype(np.float64)).astype(np.float32)
    return wt, consts


def kernel(x, spline_kernel, scale_factor, kan_bias, conv_bias):
    from concourse import bass_utils

    x = np.asarray(x, np.float32)
    spline_kernel = np.asarray(spline_kernel, np.float32)
    scale_factor = np.asarray(scale_factor, np.float32)
    kan_bias = np.asarray(kan_bias, np.float32)
    conv_bias = np.asarray(conv_bias, np.float32)

    if "nc" not in _cache:
        _cache["nc"] = _build_program()
    nc = _cache["nc"]

    wt, consts = _prep_static(spline_kernel, scale_factor,
                              kan_bias, conv_bias)

    in_maps = []
    for c in range(N_CORES):
        xc = x[c * BPC:(c + 1) * BPC]                      # (4,32,32,32)
        xtc = np.ascontiguousarray(
            xc.transpose(3, 0, 1, 2).reshape(C, BPC * PIX), np.float32
        )
        in_maps.append({"xt": xtc, "wt": wt, "consts": consts})

    res = bass_utils.run_bass_kernel_spmd(
        nc, in_maps, core_ids=list(range(N_CORES)),
        **_cache.get("run_kwargs", {})
    )
    _cache["last_result"] = res

    out = np.empty((B, HO, WO, F), np.float32)
    for c in range(N_CORES):
        yc = res.results[c]["y"]                           # (128, 3600)
        out[c * BPC:(c + 1) * BPC] = (
            yc.reshape(F, BPC, HO, WO).transpose(1, 2, 3, 0)
        )
    return out
